# revision 22
# baseline (speedup 1.0000x reference)
"""Trainium2 Bass kernel for the soft surfel rasterizer (nn_Rasterer).

Strategy: shard the PIXEL dimension across the 8 cores (2048 pixels each).
Each core rasterizes all 1024 points against its pixel slice, so the
per-pixel soft-min depth test is local to a core and no collective is
needed; the host concatenates the 8 image slices.

Per core the image slice is processed as 16 tiles of [128 pixels x 1024
points].  Math per tile (ttn == -tt throughout, so the stabilizer max and
the cull compare stay cheap):

  D   = rays . n3          (PE matmul, K=3)
  Q1  = 2 * rays . p3      (PE matmul, K=3, same weights)
  dcl = where(|D| > 1e-6, D, 1e-6)   via  max(|D|,1e-6) * sign(D + 1e-6)
  ttn = -num / dcl ;  mx = max_n ttn        (fused TTR)
  d2  = ttn*(ttn*r2 + Q1) + p2 ;  clamp 1e-12 ;  dist = sqrt
  w   = sigmoid(-400*dist + 8) * (ttn < -1e-3)
  es  = exp(50*(ttn - mx))  (+ free row-sum)          [ACT]
  S   = sum(w*es) + 1e-8*sum(es)                      (fused TTR + STT)
  prob= min(w*es/S, w)      == w * min(vis, 1)
  img = prob @ colors       (PE transpose + matmul), clamp 1.0 after sum

ACT table-bound functions (sqrt / sigmoid / exp) are swept function-major
over groups of 4 tiles and chained with ordering deps so walrus emits only
3 table loads per group.
"""

import numpy as np

RES = 128
N = 1024
NCORES = 8
PIX_PER_CORE = (RES * RES) // NCORES  # 2048
NTILES = PIX_PER_CORE // 128          # 16
GROUP = 4

DIAM = 0.04
SLOPE = 400.0
BETA = 50.0

_CACHE = {}


MAX_WAITS_PER_INST = 1


def _split_excess_waits(nc, maxw=MAX_WAITS_PER_INST):
    """The pinned walrus rejects instructions carrying more than ~2 sem
    waits.  Move excess waits onto NoOp instructions inserted immediately
    before the over-subscribed instruction on the same engine."""
    import concourse.mybir as mybir

    n_split = 0
    for fn in nc.m.functions:
        for bb in fn.blocks:
            insns = bb.instructions
            i = 0
            while i < len(insns):
                insn = insns[i]
                si = insn.sync_info
                waits = list(si.on_wait) if si is not None else []
                if len(waits) > maxw:
                    insn.sync_info = mybir.SyncInfo(
                        on_wait=waits[:maxw], on_update=list(si.on_update)
                    )
                    extra = waits[maxw:]
                    k = 0
                    while extra:
                        chunk, extra = extra[:maxw], extra[maxw:]
                        nop = mybir.InstDrain(
                            name=f"{insn.name}-wsplit{k}",
                            engine=insn.engine,
                            sync_info=mybir.SyncInfo(on_wait=chunk, on_update=[]),
                        )
                        nc.register_instruction(nop, overwrite=True)
                        insns.insert(i, nop)
                        i += 1
                        k += 1
                        n_split += 1
                i += 1
    return n_split


def _build_bass(repeat=1):
    import concourse.bass as bass
    import concourse.mybir as mybir
    from concourse.tile import TileContext
    from concourse.tile_rust import add_dep_helper


    f32 = mybir.dt.float32
    op = mybir.AluOpType
    AF = mybir.ActivationFunctionType

    nc = bass.Bass()
    raysT_d = nc.dram_tensor("raysT", [3, PIX_PER_CORE], f32, kind="ExternalInput")
    rhsc_d = nc.dram_tensor("rhsc", [3, 2 * N], f32, kind="ExternalInput")
    nmp2_d = nc.dram_tensor("nmp2", [1, 2 * N], f32, kind="ExternalInput")
    r2t_d = nc.dram_tensor("r2t", [128, NTILES], f32, kind="ExternalInput")
    colors_d = nc.dram_tensor("colors_rs", [128, 24], f32, kind="ExternalInput")
    ident_d = nc.dram_tensor("ident", [128, 128], f32, kind="ExternalInput")
    out_d = nc.dram_tensor("out", [PIX_PER_CORE, 3], f32, kind="ExternalOutput")

    act_chain = []

    def chained(inst):
        if act_chain:
            add_dep_helper(inst.ins, act_chain[-1].ins, True, "act-table-order")
        act_chain.append(inst)
        return inst

    with TileContext(nc) as tc:
        with (
            tc.tile_pool(name="consts", bufs=1) as cp,
            tc.tile_pool(name="work", bufs=2) as wp,
            tc.tile_pool(name="persist", bufs=GROUP + 1) as pp,
            tc.tile_pool(name="small", bufs=GROUP + 1) as sm,
            tc.tile_pool(name="dqp", bufs=1, space="PSUM") as dqp,
            tc.tile_pool(name="ptp", bufs=1, space="PSUM") as ptp,
            tc.tile_pool(name="colp", bufs=2, space="PSUM") as colp,
        ):
            # ---- constants into SBUF ----
            raysT = cp.tile([3, PIX_PER_CORE], f32, tag="raysT")
            nc.sync.dma_start(out=raysT[:], in_=raysT_d[:])
            rhsc = cp.tile([3, 2 * N], f32, tag="rhsc")
            nc.sync.dma_start(out=rhsc[:], in_=rhsc_d[:])
            nmp2 = cp.tile([1, 2 * N], f32, tag="nmp2")
            nc.sync.dma_start(out=nmp2[:], in_=nmp2_d[:])
            r2t = cp.tile([128, NTILES], f32, tag="r2t")
            nc.sync.dma_start(out=r2t[:], in_=r2t_d[:])
            colors = cp.tile([128, 24], f32, tag="colors")
            nc.sync.dma_start(out=colors[:], in_=colors_d[:])
            ident = cp.tile([128, 128], f32, tag="ident")
            nc.sync.dma_start(out=ident[:], in_=ident_d[:])

            ones1 = cp.tile([1, 128], f32, tag="ones1")
            nc.vector.memset(ones1[:], 1.0)
            b1e6 = cp.tile([128, 1], f32, tag="b1e6")
            nc.vector.memset(b1e6[:], 1e-6)
            b8 = cp.tile([128, 1], f32, tag="b8")
            nc.vector.memset(b8[:], SLOPE * DIAM / 2)

            # broadcast [-num | p2] across partitions via K=1 matmul
            nmp2_b = cp.tile([128, 2 * N], f32, tag="nmp2_b")
            bc = dqp.tile([128, 2 * N], f32, tag="dq")
            for j in range(4):
                nc.tensor.matmul(
                    bc[:, 512 * j : 512 * (j + 1)],
                    lhsT=ones1[:, :],
                    rhs=nmp2[:, 512 * j : 512 * (j + 1)],
                    start=True,
                    stop=True,
                )
            nc.scalar.copy(nmp2_b[:], bc[:])
            negnum_b = nmp2_b[:, :N]
            p2_b = nmp2_b[:, N:]

            for rep_g in range(repeat * (NTILES // GROUP)):
                g = rep_g % (NTILES // GROUP)
                tiles = list(range(GROUP * g, GROUP * (g + 1)))
                st = {i: {} for i in tiles}

                # ---- early phase (PSUM consumers, set-agnostic ACT) ----
                for i in tiles:
                    s = st[i]
                    dq = dqp.tile([128, 2 * N], f32, tag="dq")
                    for j in range(4):
                        nc.tensor.matmul(
                            dq[:, 512 * j : 512 * (j + 1)],
                            lhsT=raysT[:, 128 * i : 128 * (i + 1)],
                            rhs=rhsc[:, 512 * j : 512 * (j + 1)],
                            start=True,
                            stop=True,
                        )
                    D = dq[:, :N]
                    Q1 = dq[:, N:]

                    sp = pp.tile([128, N], f32, tag="sp")
                    nc.scalar.activation(sp[:], D, AF.Sign, bias=b1e6[:, 0:1])
                    q1sb = pp.tile([128, N], f32, tag="q1sb")
                    nc.scalar.copy(q1sb[:], Q1)
                    ab = pp.tile([128, N], f32, tag="chA")
                    nc.scalar.activation(ab[:], D, AF.Abs)
                    s["sp"], s["q1sb"], s["chA"] = sp, q1sb, ab

                # ---- ln/exp sweep: 1/(|D| + 1e-6) ~= clamped reciprocal, in place ----
                for i in tiles:
                    chA = st[i]["chA"]
                    chained(nc.scalar.activation(chA[:], chA[:], AF.Ln, bias=b1e6[:, 0:1]))
                    chained(nc.scalar.activation(chA[:], chA[:], AF.Exp, scale=-1.0))

                # ---- DVE phase: ttn, stabilizer max, quadratic distance ----
                for i in tiles:
                    s = st[i]
                    t1 = wp.tile([128, N], f32, tag="t1")
                    nc.vector.tensor_tensor(t1[:], s["chA"][:], negnum_b, op.mult)
                    ttn = pp.tile([128, N], f32, tag="ttn")
                    nc.vector.tensor_tensor(ttn[:], t1[:], s["sp"][:], op.mult)
                    mx = sm.tile([128, 1], f32, tag="mx")
                    nc.vector.reduce_max(mx[:], ttn[:], axis=mybir.AxisListType.X)
                    bes = sm.tile([128, 1], f32, tag="bes")
                    nc.vector.tensor_scalar_mul(bes[:], mx[:], -BETA)

                    e23 = wp.tile([128, N], f32, tag="t0")
                    nc.vector.scalar_tensor_tensor(
                        out=e23[:], in0=ttn[:], scalar=r2t[:, i : i + 1],
                        in1=s["q1sb"][:], op0=op.mult, op1=op.add,
                    )
                    e3 = wp.tile([128, N], f32, tag="t1")
                    nc.gpsimd.tensor_tensor(e3[:], e23[:], ttn[:], op.mult)
                    d2 = wp.tile([128, N], f32, tag="t2")
                    nc.gpsimd.tensor_tensor(d2[:], e3[:], p2_b, op.add)
                    chB = pp.tile([128, N], f32, tag="chB")
                    nc.vector.tensor_scalar_max(chB[:], d2[:], 1e-12)
                    s["ttn"], s["mx"], s["bes"], s["chB"] = ttn, mx, bes, chB

                # ---- sqrt sweep (in place on chB) ----
                for i in tiles:
                    chB = st[i]["chB"]
                    chained(nc.scalar.activation(chB[:], chB[:], AF.Sqrt))

                # ---- sigmoid sweep (+ cull on DVE), in place on chB ----
                for i in tiles:
                    s = st[i]
                    chB = s["chB"]
                    chained(
                        nc.scalar.activation(
                            chB[:], chB[:], AF.Sigmoid, bias=b8[:, 0:1], scale=-SLOPE
                        )
                    )
                    nc.vector.scalar_tensor_tensor(
                        out=chB[:], in0=s["ttn"][:], scalar=-1e-3, in1=chB[:],
                        op0=op.is_lt, op1=op.mult,
                    )
                    s["wcm"] = chB

                # ---- exp sweep + tail (softmin combine, prob, color matmul) ----
                for i in tiles:
                    s = st[i]
                    es = wp.tile([128, N], f32, tag="t4")
                    chained(
                        nc.scalar.activation(
                            es[:], s["ttn"][:], AF.Exp, bias=s["bes"][:, 0:1],
                            scale=BETA,
                        )
                    )
                    wces = wp.tile([128, N], f32, tag="t5")
                    Sfull = sm.tile([128, 1], f32, tag="Sfull")
                    nc.vector.scalar_tensor_tensor(
                        out=wces[:],
                        in0=s["wcm"][:],
                        scalar=1e-8,
                        in1=es[:],
                        op0=op.add,
                        op1=op.mult,
                        accum_out=Sfull[:],
                    )
                    rS = sm.tile([128, 1], f32, tag="rS")
                    nc.vector.reciprocal(rS[:], Sfull[:])
                    prob = wp.tile([128, N], f32, tag="tProb")
                    nc.vector.scalar_tensor_tensor(
                        out=prob[:], in0=wces[:], scalar=rS[:, 0:1],
                        in1=s["wcm"][:], op0=op.mult, op1=op.min,
                    )

                    # ---- color reduction: transpose prob, matmul with colors ----
                    probT_ps = ptp.tile([128, N], f32, tag="pt")
                    for c in range(8):
                        nc.tensor.transpose(
                            probT_ps[:, 128 * c : 128 * (c + 1)],
                            prob[:, 128 * c : 128 * (c + 1)],
                            ident[:],
                        )
                    probT = wp.tile([128, N], f32, tag="tProbT")
                    nc.scalar.copy(probT[:, :512], probT_ps[:, :512])
                    nc.scalar.copy(probT[:, 512:], probT_ps[:, 512:])
                    color_ps = colp.tile([128, 3], f32, tag="col")
                    for c in range(8):
                        nc.tensor.matmul(
                            color_ps[:],
                            lhsT=probT[:, 128 * c : 128 * (c + 1)],
                            rhs=colors[:, 3 * c : 3 * (c + 1)],
                            start=(c == 0),
                            stop=(c == 7),
                        )
                    outc = sm.tile([128, 3], f32, tag="outc")
                    nc.vector.tensor_scalar_min(outc[:], color_ps[:], 1.0)
                    nc.sync.dma_start(
                        out=out_d[128 * i : 128 * (i + 1), :], in_=outc[:]
                    )
    _split_excess_waits(nc)
    return nc


def _get_nc(repeat=1):
    key = ("nc", repeat)
    if key not in _CACHE:
        _CACHE[key] = _build_bass(repeat)
    return _CACHE[key]


def _host_precompute(coords, normals, colors, camera_matrix, K):
    f4 = np.float32
    coords = np.asarray(coords, f4)
    normals = np.asarray(normals, f4)
    colors = np.asarray(colors, f4)
    camera_matrix = np.asarray(camera_matrix, f4)
    if K is None:
        diag_px = float(np.hypot(RES, RES))
        f = f4(70.0 / 20.0 * diag_px)
        K = np.array([[f, 0.0, RES / 2], [0.0, f, RES / 2], [0.0, 0.0, 1.0]], f4)
    else:
        K = np.asarray(K, f4)

    q = camera_matrix[:4]
    q = q / f4(np.linalg.norm(q))
    w, x, y, z = q
    R = np.array(
        [
            [1 - 2 * (y * y + z * z), 2 * (x * y - w * z), 2 * (x * z + w * y)],
            [2 * (x * y + w * z), 1 - 2 * (x * x + z * z), 2 * (y * z - w * x)],
            [2 * (x * z - w * y), 2 * (y * z + w * x), 1 - 2 * (x * x + y * y)],
        ],
        f4,
    )
    t = camera_matrix[4:]
    p3 = (coords @ R.T + t).astype(f4)
    n3 = (normals @ R.T).astype(f4)

    yy, xx = np.mgrid[0:RES, 0:RES]
    pix = np.stack([xx.ravel(), yy.ravel(), np.ones(RES * RES)], -1).astype(f4)
    Kinv = np.linalg.inv(K.astype(np.float64)).astype(f4)
    rays = (pix @ Kinv.T).astype(f4)

    num = np.sum(p3 * n3, -1)
    p2 = np.sum(p3 * p3, -1)
    r2 = np.sum(rays * rays, -1)

    rhsc = np.concatenate([n3.T, (2.0 * p3).T], axis=1).astype(f4)          # [3, 2N]
    nmp2 = np.concatenate([-num, p2])[None, :].astype(f4)                    # [1, 2N]
    colors_rs = (
        colors.reshape(8, 128, 3).transpose(1, 0, 2).reshape(128, 24).astype(f4)
    )
    ident = np.eye(128, dtype=f4)

    in_maps = []
    for c in range(NCORES):
        sl = slice(c * PIX_PER_CORE, (c + 1) * PIX_PER_CORE)
        in_maps.append(
            {
                "raysT": np.ascontiguousarray(rays[sl].T),
                "rhsc": rhsc,
                "nmp2": nmp2,
                "r2t": np.ascontiguousarray(r2[sl].reshape(NTILES, 128).T),
                "colors_rs": colors_rs,
                "ident": ident,
            }
        )
    return in_maps


def kernel(coords, normals, colors, camera_matrix, K=None, **_ignored):
    from concourse.bass_utils import run_bass_kernel_spmd

    in_maps = _host_precompute(coords, normals, colors, camera_matrix, K)
    nc = _get_nc()
    res = run_bass_kernel_spmd(nc, in_maps, core_ids=list(range(NCORES)))
    out = np.concatenate([res.results[c]["out"] for c in range(NCORES)], axis=0)
    return np.ascontiguousarray(out.T.reshape(3, RES, RES)).astype(np.float32)

